# revision 1
# baseline (speedup 1.0000x reference)
"""TRN2 Bass kernel for nn_GQA_22436909154699.

Reference math: softmax over a size-1 axis is identically 1.0, so
    out[b,l,g,h,:] = v[b,l,g,:]          (v = v-half of x @ Wkv + bkv)
The q projection (x @ Wq) never affects the output.  The kernel computes
    res = x @ Wv + bv                    (K=2048, N=256, fp32)
data-parallel over tokens across 8 NeuronCores (2048 tokens each), then
broadcasts each group's 64-wide v vector across the 8 heads-per-group
on-chip before a contiguous store.

Host-side prep per core:
  - x is pre-transposed into [tt, p, ko, t] tiles so that every DMA lands
    contiguous per SBUF partition (p = k mod 128 on partitions).
  - Wv columns of Wkv are sliced out and repacked to [128, ko, 256].
  - bias is pre-broadcast to the [128, g, h, d] output layout.
"""

import os

import numpy as np

# Problem constants (hardcoded; harness runs kernel.py standalone).
B, L, E = 4, 4096, 2048
G, HPG, D = 4, 8, 64
NV = G * D  # 256 v-columns
NCORES = 8
TOK = B * L  # 16384 tokens
TPC = TOK // NCORES  # 2048 tokens per core
TT = TPC // 128  # 16 token tiles per core
KO = E // 128  # 16 contraction tiles

_CACHE: dict = {}
LAST_RESULTS = None


def _build(mm_dtype_name: str):
    import concourse.bacc as bacc
    import concourse.mybir as mybir
    import concourse.tile as tile

    F32 = mybir.dt.float32
    MM_DT = getattr(mybir.dt, mm_dtype_name)

    nc = bacc.Bacc(
        "TRN2", target_bir_lowering=False, debug=False, num_devices=NCORES
    )
    xt_d = nc.dram_tensor("xt", [TT, 128, KO, 128], F32, kind="ExternalInput")
    wv_d = nc.dram_tensor("wv", [128, KO, NV], F32, kind="ExternalInput")
    bias_d = nc.dram_tensor("bias", [128, G, HPG, D], F32, kind="ExternalInput")
    out_d = nc.dram_tensor("out", [TPC, E], F32, kind="ExternalOutput")

    with tile.TileContext(nc) as tc:
        with (
            tc.tile_pool(name="const", bufs=1) as cpool,
            tc.tile_pool(name="xin", bufs=4) as xpool,
            tc.tile_pool(name="obuf", bufs=4) as opool,
            tc.tile_pool(name="ps", bufs=4, space="PSUM") as ppool,
        ):
            wv_sb = cpool.tile([128, KO, NV], F32)
            nc.sync.dma_start(wv_sb[:], wv_d[:])
            bias_sb = cpool.tile([128, G, HPG, D], F32)
            nc.sync.dma_start(bias_sb[:], bias_d[:])

            for tt in range(TT):
                xin = xpool.tile([128, KO, 128], F32, tag="xin")
                nc.sync.dma_start(xin[:], xt_d[tt])
                ps = ppool.tile([128, NV], F32, tag="ps")
                for k in range(KO):
                    lhsT = xin[:, k, :]
                    rhs = wv_sb[:, k, :]
                    if MM_DT != F32:
                        lhsT = lhsT.bitcast(MM_DT)
                        rhs = rhs.bitcast(MM_DT)
                    nc.tensor.matmul(
                        ps[:],
                        lhsT=lhsT,
                        rhs=rhs,
                        start=(k == 0),
                        stop=(k == KO - 1),
                    )
                ot = opool.tile([128, G, HPG, D], F32, tag="ot")
                ps_g = ps[:].rearrange("p (g d) -> p g d", g=G)
                for g in range(G):
                    nc.vector.tensor_add(
                        ot[:, g],
                        ps_g[:, g, None, :].to_broadcast([128, HPG, D]),
                        bias_sb[:, g],
                    )
                nc.sync.dma_start(
                    out_d[tt * 128 : (tt + 1) * 128, :],
                    ot[:].rearrange("p g h d -> p (g h d)"),
                )
    nc.compile()
    return nc


def _get_nc():
    mm = os.environ.get("GQA_MM_DT", "float32")
    key = ("nc", mm)
    if key not in _CACHE:
        _CACHE[key] = _build(mm)
    return _CACHE[key]


def _prep_inputs(x, Wkv, bkv):
    x = np.ascontiguousarray(np.asarray(x, dtype=np.float32))
    Wkv = np.asarray(Wkv, dtype=np.float32)
    bkv = np.asarray(bkv, dtype=np.float32)

    # v-columns of the kv projection: Wkv reshaped (E, G, 2, D), kv index 1.
    wv = Wkv.reshape(E, G, 2, D)[:, :, 1, :].reshape(E, NV)  # (2048, 256)
    bv = bkv.reshape(G, 2, D)[:, 1, :]  # (G, D)

    wv_dev = np.ascontiguousarray(
        wv.reshape(KO, 128, NV).transpose(1, 0, 2)
    )  # (128, KO, NV): wv_dev[p, ko, n] = Wv[ko*128+p, n]
    bias_dev = np.ascontiguousarray(
        np.broadcast_to(bv[None, :, None, :], (128, G, HPG, D))
    ).astype(np.float32)

    # x tokens: (core, tt, t, ko, p) -> per-core [tt, p, ko, t]
    xt = x.reshape(NCORES, TT, 128, KO, 128)
    xt = np.ascontiguousarray(xt.transpose(0, 1, 4, 3, 2))
    return xt, wv_dev, bias_dev


def kernel(x, Wq, bq, Wkv, bkv):
    global LAST_RESULTS
    from concourse.bass_utils import run_bass_kernel_spmd

    nc = _get_nc()
    xt, wv_dev, bias_dev = _prep_inputs(x, Wkv, bkv)
    in_maps = [
        {"xt": xt[c], "wv": wv_dev, "bias": bias_dev} for c in range(NCORES)
    ]
    res = run_bass_kernel_spmd(nc, in_maps, core_ids=list(range(NCORES)))
    LAST_RESULTS = res
    out = np.concatenate(
        [res.results[c]["out"] for c in range(NCORES)], axis=0
    )
    return np.ascontiguousarray(out.reshape(B, L, E).astype(np.float32))
